# revision 1
# baseline (speedup 1.0000x reference)
"""nn_Linear8bit on 8 TRN2 NeuronCores — column-parallel (tensor-parallel on out_features).

out[m, n] = sum_k x[m, k] * wq[n, k] * scale[n] + bias[n]
  x: [2, 512, 4096] f32, wq: [16384, 4096] int32 (int8-valued), scale/bias: [16384] f32

Sharding: W/scale/bias row-sharded 2048/core; x replicated (fed k-major as part of
layout prep); no collectives.

Per-core dataflow:
  - x.T (k-major f32) -> gpsimd cast-DMA f32->bf16 straight into resident SBUF
    tiles xT[kp, kt, m]  (contraction dim on partitions).
  - per n-tile (128 rows of W): gpsimd cast-DMA int32->bf16 (SDMA casts in the
    datapath), xbar DMA-transpose (Sync engine, transposes only -> no xbar/copy
    mode transitions) to wT[kp, kt, n].
  - 2 x 32 accumulating matmuls per n-tile (k-inner, one PSUM bank per 512-token
    chunk), PSUM f32 evicted via one DVE tensor_scalar (x*scale + bias, both
    per-partition scalars), output written as out.T [2048, 1024] f32 on Scalar
    HWDGE (keeps Sync xbar-only).
  - host: concat core outputs along n, transpose to [1024, 16384].
"""

import numpy as np

import concourse.tile as tile
from concourse import bacc, mybir
from concourse.bass_utils import run_bass_kernel_spmd

B, S, K, N = 2, 512, 4096, 16384
M = B * S              # 1024 tokens
NCORES = 8
NSH = N // NCORES      # 2048 out-features per core
P = 128
KT = K // P            # 32 k-tiles
NT = NSH // P          # 16 n-tiles per core
MCW = 512              # moving free dim per matmul (= one PSUM bank of f32)
MCH = M // MCW         # 2 token chunks
XG = 8                 # x load groups (4 k-tiles per DMA)


def build(w_bufs: int = 5, psum_bufs: int = 3):
    nc = bacc.Bacc("TRN2", target_bir_lowering=False, debug=False)
    xT_d = nc.dram_tensor("xT", [K, M], mybir.dt.float32, kind="ExternalInput")
    w_d = nc.dram_tensor("wq", [NSH, K], mybir.dt.int32, kind="ExternalInput")
    s_d = nc.dram_tensor("scale", [NSH, 1], mybir.dt.float32, kind="ExternalInput")
    b_d = nc.dram_tensor("bias", [NSH, 1], mybir.dt.float32, kind="ExternalInput")
    o_d = nc.dram_tensor("outT", [NSH, M], mybir.dt.float32, kind="ExternalOutput")

    kt_per_g = KT // XG
    with tile.TileContext(nc) as tc:
        with (
            tc.tile_pool(name="xT_pool", bufs=1) as xT_pool,
            tc.tile_pool(name="xstage", bufs=2) as xstage_pool,
            tc.tile_pool(name="wstage", bufs=w_bufs) as wstage_pool,
            tc.tile_pool(name="wT_pool", bufs=w_bufs) as wT_pool,
            tc.tile_pool(name="small", bufs=4) as small_pool,
            tc.tile_pool(name="osb", bufs=4) as osb_pool,
            tc.tile_pool(name="psum", bufs=psum_bufs, space="PSUM") as psum_pool,
        ):
            # x: f32 load on Scalar HWDGE (keeps the one SWDGE ring free for W
            # casts), DVE cast f32->bf16 into the resident k-major layout.
            # One tile per 4-k-tile group so matmuls depend only on the groups
            # they actually read, not on the whole x load.
            xTs = []
            for g in range(XG):
                xt_g = xT_pool.tile(
                    [P, kt_per_g, M], mybir.dt.bfloat16, name=f"xT{g}", tag=f"xT{g}"
                )
                xstg = xstage_pool.tile(
                    [P, kt_per_g, M], mybir.dt.float32, tag="xstg"
                )
                nc.scalar.dma_start(
                    out=xstg[:],
                    in_=xT_d.ap()[g * kt_per_g * P:(g + 1) * kt_per_g * P, :].rearrange(
                        "(kt p) m -> p kt m", p=P
                    ),
                )
                nc.vector.tensor_copy(out=xt_g[:], in_=xstg[:])
                xTs.append(xt_g)

            for nt in range(NT):
                w_sb = wstage_pool.tile([P, K], mybir.dt.bfloat16, tag="w_sb")
                nc.gpsimd.dma_start(out=w_sb[:], in_=w_d.ap()[nt * P:(nt + 1) * P, :])
                wT = wT_pool.tile([P, KT, P], mybir.dt.bfloat16, tag="wT")
                nc.sync.dma_start(out=wT[:], in_=w_sb[:], transpose=True)

                s_sb = small_pool.tile([P, 1], mybir.dt.float32, tag="s_sb")
                nc.scalar.dma_start(out=s_sb[:], in_=s_d.ap()[nt * P:(nt + 1) * P, :])
                b_sb = small_pool.tile([P, 1], mybir.dt.float32, tag="b_sb")
                nc.scalar.dma_start(out=b_sb[:], in_=b_d.ap()[nt * P:(nt + 1) * P, :])

                for c in range(MCH):
                    ps = psum_pool.tile(
                        [P, MCW], mybir.dt.float32, name=f"ps{c}", tag=f"ps{c}"
                    )
                    # k-inner: 32 back-to-back accumulating matmuls on one bank,
                    # 2D contiguous moving operand.
                    for kt in range(KT):
                        nc.tensor.matmul(
                            ps[:],
                            wT[:, kt, :],
                            xTs[kt // kt_per_g][:, kt % kt_per_g, c * MCW:(c + 1) * MCW],
                            start=(kt == 0),
                            stop=(kt == KT - 1),
                        )
                    o_sb = osb_pool.tile([P, MCW], mybir.dt.float32, tag="o_sb")
                    nc.vector.tensor_scalar(
                        out=o_sb[:],
                        in0=ps[:],
                        scalar1=s_sb[:],
                        scalar2=b_sb[:],
                        op0=mybir.AluOpType.mult,
                        op1=mybir.AluOpType.add,
                    )
                    nc.scalar.dma_start(
                        out=o_d.ap()[nt * P:(nt + 1) * P, c * MCW:(c + 1) * MCW],
                        in_=o_sb[:],
                    )
    nc.compile()
    return nc


def make_in_maps(x, weight_quant, scale, bias):
    x2T = np.ascontiguousarray(
        np.asarray(x, dtype=np.float32).reshape(M, K).T
    )  # [K, M] k-major replica
    scale = np.asarray(scale, dtype=np.float32).reshape(N, 1)
    bias = np.asarray(bias, dtype=np.float32).reshape(N, 1)
    wq = np.asarray(weight_quant, dtype=np.int32)
    in_maps = []
    for i in range(NCORES):
        sl = slice(i * NSH, (i + 1) * NSH)
        in_maps.append({
            "xT": x2T,
            "wq": np.ascontiguousarray(wq[sl]),
            "scale": np.ascontiguousarray(scale[sl]),
            "bias": np.ascontiguousarray(bias[sl]),
        })
    return in_maps


def gather_output(results):
    outT = np.concatenate([np.asarray(r["outT"]) for r in results], axis=0)  # [N, M]
    return np.ascontiguousarray(outT.T).reshape(B, S, N).astype(np.float32, copy=False)


def kernel(x, weight_quant, scale, bias):
    nc = build()
    in_maps = make_in_maps(x, weight_quant, scale, bias)
    res = run_bass_kernel_spmd(nc, in_maps, core_ids=list(range(NCORES)))
    return gather_output(res.results)


if __name__ == "__main__":
    rng = np.random.default_rng(0)
    x = rng.standard_normal((B, S, K), dtype=np.float32)
    wq = rng.integers(-128, 128, size=(N, K), dtype=np.int64).astype(np.int32)
    scale = rng.uniform(0.001, 0.02, size=(N,)).astype(np.float32)
    bias = rng.standard_normal((N,), dtype=np.float32)
    out = kernel(x=x, weight_quant=wq, scale=scale, bias=bias)
    w = wq.astype(np.float32) * scale[:, None]
    exp = x.reshape(M, K) @ w.T + bias
    err = np.abs(out.reshape(M, N) - exp).max() / np.abs(exp).max()
    print("self-check rel err:", err)



# revision 4
# speedup vs baseline: 1.4357x; 1.4357x over previous
"""nn_Linear8bit on 8 TRN2 NeuronCores — column-parallel (tensor-parallel on out_features).

out[m, n] = sum_k x[m, k] * wq[n, k] * scale[n] + bias[n]
  x: [2, 512, 4096] f32, wq: [16384, 4096] int32 (int8-valued), scale/bias: [16384] f32

Sharding: W/scale/bias row-sharded 2048/core; x replicated (fed k-major). No collectives.

Host prep (pure layout/bit repack, no arithmetic):
  - x -> x.T [K, M] f32 (k-major replica).
  - wq (int8-valued int32) -> int8, transposed+swizzled to [nt*128, kt, n] so each
    n-tile's stationary block DMAs as contiguous 4KB partition lines.
  - scale/bias -> [128, 16] (partition-major per n-tile).

Per-core dataflow (all HWDGE, no SWDGE cast path, no on-chip transposes):
  - x: f32 DMA (sync ring) in 8 k-groups -> DVE cast f32->bf16 into resident
    xT[kp, kt, m] tiles (contraction on partitions).
  - W: int8 DMA (sync ring) per n-tile -> DVE cast int8->bf16 (int8 values exact
    in bf16) -> wT[kp, kt, n].
  - Startup phase: first 4 n-tiles processed k-group-major with 8 live PSUM
    accumulators so the PE starts ~6us in and stays busy while x streams.
  - Steady phase: remaining 12 n-tiles k-inner, ch-inner (one stationary per
    (nt,kt) feeds both 512-token chunks), PSUM evicted via one DVE tensor_scalar
    (x*scale + bias, per-partition scalars), outputs stored as out.T f32 on the
    scalar HWDGE ring.
  - host: concat core outputs along n, transpose to [1024, 16384].
"""

import numpy as np

import concourse.tile as tile
from concourse import bacc, mybir
from concourse.bass_utils import run_bass_kernel_spmd

B, S, K, N = 2, 512, 4096, 16384
M = B * S              # 1024 tokens
NCORES = 8
NSH = N // NCORES      # 2048 out-features per core
P = 128
KT = K // P            # 32 k-tiles
NT = NSH // P          # 16 n-tiles per core
MCW = 512              # moving free dim per matmul (= one PSUM bank of f32)
MCH = M // MCW         # 2 token chunks
XG = 8                 # x load groups (4 k-tiles per DMA)
KTG = KT // XG         # k-tiles per x group
NT_A = 4               # n-tiles processed in the k-group-major startup phase


def build(w_bufs: int = 4, x_bufs: int = 4, psum_bufs: int = 8):
    nc = bacc.Bacc("TRN2", target_bir_lowering=False, debug=False)
    xT_d = nc.dram_tensor("xT", [K, M], mybir.dt.float32, kind="ExternalInput")
    w_d = nc.dram_tensor("wq", [NT * P, KT, P], mybir.dt.int8, kind="ExternalInput")
    s_d = nc.dram_tensor("scale", [P, NT], mybir.dt.float32, kind="ExternalInput")
    b_d = nc.dram_tensor("bias", [P, NT], mybir.dt.float32, kind="ExternalInput")
    o_d = nc.dram_tensor("outT", [NSH, M], mybir.dt.float32, kind="ExternalOutput")

    with tile.TileContext(nc) as tc:
        with (
            tc.tile_pool(name="xT_pool", bufs=1) as xT_pool,
            tc.tile_pool(name="xstage", bufs=x_bufs) as xstage_pool,
            tc.tile_pool(name="w8", bufs=w_bufs) as w8_pool,
            tc.tile_pool(name="wT_pool", bufs=w_bufs) as wT_pool,
            tc.tile_pool(name="small", bufs=2) as small_pool,
            tc.tile_pool(name="osb", bufs=4) as osb_pool,
            tc.tile_pool(name="psum", bufs=psum_bufs, space="PSUM") as psum_pool,
        ):
            s_sb = small_pool.tile([P, NT], mybir.dt.float32, tag="s_sb")
            nc.scalar.dma_start(out=s_sb[:], in_=s_d.ap())
            b_sb = small_pool.tile([P, NT], mybir.dt.float32, tag="b_sb")
            nc.scalar.dma_start(out=b_sb[:], in_=b_d.ap())

            def load_w(nt):
                w8 = w8_pool.tile([P, KT, P], mybir.dt.int8, tag="w8")
                nc.sync.dma_start(out=w8[:], in_=w_d.ap()[nt * P:(nt + 1) * P])
                wT = wT_pool.tile([P, KT, P], mybir.dt.bfloat16, tag="wT")
                nc.vector.tensor_copy(out=wT[:], in_=w8[:])
                return wT

            def load_x(g):
                xstg = xstage_pool.tile([P, KTG, M], mybir.dt.float32, tag="xstg")
                nc.sync.dma_start(
                    out=xstg[:],
                    in_=xT_d.ap()[g * KTG * P:(g + 1) * KTG * P, :].rearrange(
                        "(kt p) m -> p kt m", p=P
                    ),
                )
                xt = xT_pool.tile(
                    [P, KTG, M], mybir.dt.bfloat16, name=f"xT{g}", tag=f"xT{g}"
                )
                nc.vector.tensor_copy(out=xt[:], in_=xstg[:])
                return xt

            def evict(nt, c, ps):
                o_sb = osb_pool.tile([P, MCW], mybir.dt.float32, tag="o_sb")
                nc.vector.tensor_scalar(
                    out=o_sb[:],
                    in0=ps[:],
                    scalar1=s_sb[:, nt:nt + 1],
                    scalar2=b_sb[:, nt:nt + 1],
                    op0=mybir.AluOpType.mult,
                    op1=mybir.AluOpType.add,
                )
                nc.scalar.dma_start(
                    out=o_d.ap()[nt * P:(nt + 1) * P, c * MCW:(c + 1) * MCW],
                    in_=o_sb[:],
                )

            # ---- startup: W tiles 0..NT_A-1 + x groups, k-group-major matmuls
            wTs = {}
            xTs = []
            # interleave the first W loads with the first x groups on the sync
            # ring so neither delays the other's first arrival.
            for i in range(NT_A):
                wTs[i] = load_w(i)
                xTs.append(load_x(i))
            for g in range(NT_A, XG):
                xTs.append(load_x(g))

            psA = [
                [
                    psum_pool.tile(
                        [P, MCW], mybir.dt.float32, name=f"psA{nt}_{c}", tag="ps"
                    )
                    for c in range(MCH)
                ]
                for nt in range(NT_A)
            ]
            for g in range(XG):
                for nt in range(NT_A):
                    for kti in range(KTG):
                        kt = g * KTG + kti
                        for c in range(MCH):
                            nc.tensor.matmul(
                                psA[nt][c][:],
                                wTs[nt][:, kt, :],
                                xTs[g][:, kti, c * MCW:(c + 1) * MCW],
                                start=(kt == 0),
                                stop=(kt == KT - 1),
                            )
            for nt in range(NT_A):
                for c in range(MCH):
                    evict(nt, c, psA[nt][c])

            # ---- steady state: remaining n-tiles, k-inner / ch-inner
            for nt in range(NT_A, NT):
                wT = load_w(nt)
                ps = [
                    psum_pool.tile(
                        [P, MCW], mybir.dt.float32, name=f"ps{nt}_{c}", tag="ps"
                    )
                    for c in range(MCH)
                ]
                for kt in range(KT):
                    g, kti = divmod(kt, KTG)
                    for c in range(MCH):
                        nc.tensor.matmul(
                            ps[c][:],
                            wT[:, kt, :],
                            xTs[g][:, kti, c * MCW:(c + 1) * MCW],
                            start=(kt == 0),
                            stop=(kt == KT - 1),
                        )
                for c in range(MCH):
                    evict(nt, c, ps[c])

    nc.compile()
    return nc


def make_in_maps(x, weight_quant, scale, bias):
    x2T = np.ascontiguousarray(
        np.asarray(x, dtype=np.float32).reshape(M, K).T
    )  # [K, M] k-major replica
    wq = np.asarray(weight_quant, dtype=np.int32).astype(np.int8)  # int8-valued
    scale = np.asarray(scale, dtype=np.float32)
    bias = np.asarray(bias, dtype=np.float32)
    in_maps = []
    for i in range(NCORES):
        sl = slice(i * NSH, (i + 1) * NSH)
        # [nsh, k] -> [nt, n, kt, p] -> [nt, p, kt, n] -> [nt*p, kt, n]
        w_sw = np.ascontiguousarray(
            wq[sl].reshape(NT, P, KT, P).transpose(0, 3, 2, 1)
        ).reshape(NT * P, KT, P)
        in_maps.append({
            "xT": x2T,
            "wq": w_sw,
            "scale": np.ascontiguousarray(scale[sl].reshape(NT, P).T),
            "bias": np.ascontiguousarray(bias[sl].reshape(NT, P).T),
        })
    return in_maps


def gather_output(results):
    outT = np.concatenate([np.asarray(r["outT"]) for r in results], axis=0)  # [N, M]
    return np.ascontiguousarray(outT.T).reshape(B, S, N).astype(np.float32, copy=False)


def kernel(x, weight_quant, scale, bias):
    nc = build()
    in_maps = make_in_maps(x, weight_quant, scale, bias)
    res = run_bass_kernel_spmd(nc, in_maps, core_ids=list(range(NCORES)))
    return gather_output(res.results)


if __name__ == "__main__":
    rng = np.random.default_rng(0)
    x = rng.standard_normal((B, S, K), dtype=np.float32)
    wq = rng.integers(-128, 128, size=(N, K), dtype=np.int64).astype(np.int32)
    scale = rng.uniform(0.001, 0.02, size=(N,)).astype(np.float32)
    bias = rng.standard_normal((N,), dtype=np.float32)
    out = kernel(x=x, weight_quant=wq, scale=scale, bias=bias)
    w = wq.astype(np.float32) * scale[:, None]
    exp = x.reshape(M, K) @ w.T + bias
    err = np.abs(out.reshape(M, N) - exp).max() / np.abs(exp).max()
    print("self-check rel err:", err)


# revision 8
# speedup vs baseline: 1.4447x; 1.0062x over previous
"""nn_Linear8bit on 8 TRN2 NeuronCores — column-parallel (tensor-parallel on out_features).

out[m, n] = sum_k x[m, k] * wq[n, k] * scale[n] + bias[n]
  x: [2, 512, 4096] f32, wq: [16384, 4096] int32 (int8-valued), scale/bias: [16384] f32

Sharding: W/scale/bias row-sharded 2048/core; x replicated (fed k-major). No collectives.

Host prep (pure layout/bit repack, no arithmetic):
  - x -> x.T [K, M] f32 (k-major replica).
  - wq (int8-valued int32) -> int8, transposed+swizzled to [nt*128, kt, n] so each
    n-tile's stationary block DMAs as contiguous 4KB partition lines.
  - scale/bias -> [128, 16] (partition-major per n-tile).

Per-core dataflow (all HWDGE, no SWDGE, no on-chip transposes):
  - x: f32 DMA on the ACT HWDGE ring (its own ring, fine-grained first pieces so
    the first k-tile lands ~10.5us) -> DVE cast f32->bf16 into resident
    xT[kp, kt, m] tiles (contraction on partitions).
  - W: int8 DMA on the SP HWDGE ring per n-tile -> DVE cast int8->bf16 (int8
    values exact in bf16); first 4 tiles cast in two pieces (kt 0..7 / 8..31)
    so the PE's first stationaries are ready early.
  - ~12 dummy warm-up matmuls on a memset tile run during the initial DMA dead
    time so the PE_HAM clock-gate is at 8/8 when real matmuls start.
  - Startup phase: first 4 n-tiles processed k-group-major with 8 live PSUM
    accumulators while x streams in; steady phase: remaining 12 n-tiles k-inner,
    ch-inner (one stationary per (nt,kt) feeds both 512-token chunks).
  - PSUM evicted via one DVE tensor_scalar (x*scale + bias, per-partition
    scalars); outputs stored as out.T f32 on the SP ring.
  - host: concat core outputs along n, transpose to [1024, 16384].
"""

import numpy as np

import concourse.tile as tile
from concourse import bacc, mybir
from concourse.bass_utils import run_bass_kernel_spmd

B, S, K, N = 2, 512, 4096, 16384
M = B * S              # 1024 tokens
NCORES = 8
NSH = N // NCORES      # 2048 out-features per core
P = 128
KT = K // P            # 32 k-tiles
NT = NSH // P          # 16 n-tiles per core
MCW = 512              # moving free dim per matmul (= one PSUM bank of f32)
MCH = M // MCW         # 2 token chunks
NT_A = 4               # n-tiles processed in the k-group-major startup phase
WSPL = 8               # first-phase W tiles cast in (kt<WSPL, kt>=WSPL) pieces
NDUMMY = 12            # warm-up matmuls

# x load piece sizes in k-tiles: small first pieces for fast PE start.
KGS = [1] * 6 + [2] * 13
assert sum(KGS) == KT
KG_START = np.cumsum([0] + KGS).tolist()   # group -> first kt
XG = len(KGS)


def _group_of(kt):
    for g in range(XG):
        if KG_START[g] <= kt < KG_START[g + 1]:
            return g, kt - KG_START[g]
    raise AssertionError


def build(w_bufs: int = 4, x_bufs: int = 4, psum_bufs: int = 8):
    nc = bacc.Bacc("TRN2", target_bir_lowering=False, debug=False)
    xT_d = nc.dram_tensor("xT", [K, M], mybir.dt.float32, kind="ExternalInput")
    w_d = nc.dram_tensor("wq", [NT * P, KT, P], mybir.dt.int8, kind="ExternalInput")
    s_d = nc.dram_tensor("scale", [P, NT], mybir.dt.float32, kind="ExternalInput")
    b_d = nc.dram_tensor("bias", [P, NT], mybir.dt.float32, kind="ExternalInput")
    o_d = nc.dram_tensor("outT", [NSH, M], mybir.dt.float32, kind="ExternalOutput")

    with tile.TileContext(nc) as tc:
        with (
            tc.tile_pool(name="xT_pool", bufs=1) as xT_pool,
            tc.tile_pool(name="xstage", bufs=x_bufs) as xstage_pool,
            tc.tile_pool(name="w8", bufs=w_bufs) as w8_pool,
            tc.tile_pool(name="wT_pool", bufs=w_bufs) as wT_pool,
            tc.tile_pool(name="wTa_pool", bufs=1) as wTa_pool,
            tc.tile_pool(name="wTb_pool", bufs=1) as wTb_pool,
            tc.tile_pool(name="small", bufs=2) as small_pool,
            tc.tile_pool(name="osb", bufs=4) as osb_pool,
            tc.tile_pool(name="psum", bufs=psum_bufs, space="PSUM") as psum_pool,
        ):
            # ---- PE warm-up: dummy matmuls on a zeroed tile during DMA dead time
            dummy = small_pool.tile([P, MCW], mybir.dt.bfloat16, tag="dummy")
            nc.vector.memset(dummy[:], 0.0)

            psA = [
                [
                    psum_pool.tile(
                        [P, MCW], mybir.dt.float32, name=f"psA{nt}_{c}", tag="ps"
                    )
                    for c in range(MCH)
                ]
                for nt in range(NT_A)
            ]
            for i in range(NDUMMY):
                nc.tensor.matmul(
                    psA[0][0][:], dummy[:, 0:P], dummy[:], start=True, stop=True
                )

            # ---- startup DMAs: W tiles 0..NT_A-1 on the SP ring, x pieces on
            # the ACT ring (independent rings, transfers overlap).
            w8s = {}
            for nt in range(NT_A):
                w8s[nt] = w8_pool.tile(
                    [P, KT, P], mybir.dt.int8, name=f"w8_{nt}", tag="w8"
                )
                nc.sync.dma_start(
                    out=w8s[nt][:], in_=w_d.ap()[nt * P:(nt + 1) * P]
                )
            xstgs = []
            for g in range(XG):
                xstg = xstage_pool.tile(
                    [P, KGS[g], M], mybir.dt.float32, name=f"xstg{g}", tag="xstg"
                )
                nc.scalar.dma_start(
                    out=xstg[:],
                    in_=xT_d.ap()[
                        KG_START[g] * P:KG_START[g + 1] * P, :
                    ].rearrange("(kt p) m -> p kt m", p=P),
                )
                xstgs.append(xstg)
            s_sb = small_pool.tile([P, NT], mybir.dt.float32, tag="s_sb")
            nc.sync.dma_start(out=s_sb[:], in_=s_d.ap())
            b_sb = small_pool.tile([P, NT], mybir.dt.float32, tag="b_sb")
            nc.sync.dma_start(out=b_sb[:], in_=b_d.ap())

            # ---- DVE cast order: W a-pieces and first x pieces interleaved so
            # neither blocks the other's earliest consumer.
            wTa = {}
            wTb = {}
            xTs = [None] * XG

            def cast_x(g):
                xt = xT_pool.tile(
                    [P, KGS[g], M], mybir.dt.bfloat16, name=f"xT{g}", tag=f"xT{g}"
                )
                nc.vector.tensor_copy(out=xt[:], in_=xstgs[g][:])
                xTs[g] = xt

            for nt in range(NT_A):
                wTa[nt] = wTa_pool.tile(
                    [P, WSPL, P], mybir.dt.bfloat16, name=f"wTa{nt}", tag=f"wTa{nt}"
                )
                nc.vector.tensor_copy(out=wTa[nt][:], in_=w8s[nt][:, 0:WSPL, :])
                cast_x(nt)
            for nt in range(NT_A):
                wTb[nt] = wTb_pool.tile(
                    [P, KT - WSPL, P], mybir.dt.bfloat16, name=f"wTb{nt}",
                    tag=f"wTb{nt}"
                )
                nc.vector.tensor_copy(out=wTb[nt][:], in_=w8s[nt][:, WSPL:KT, :])
                cast_x(NT_A + nt)
            for g in range(2 * NT_A, XG):
                cast_x(g)

            def stationary(nt, kt):
                if nt < NT_A:
                    if kt < WSPL:
                        return wTa[nt][:, kt, :]
                    return wTb[nt][:, kt - WSPL, :]
                return wTs[nt][:, kt, :]

            # ---- phase B W prefetch (nt NT_A..NT_A+3): DMA now, cast before
            # the phase-A evicts enter the DVE queue (in-order engine).
            wTs = {}

            def load_w_full(nt):
                w8 = w8_pool.tile([P, KT, P], mybir.dt.int8, name=f"w8_{nt}", tag="w8")
                nc.sync.dma_start(out=w8[:], in_=w_d.ap()[nt * P:(nt + 1) * P])
                wT = wT_pool.tile(
                    [P, KT, P], mybir.dt.bfloat16, name=f"wT{nt}", tag="wT"
                )
                nc.vector.tensor_copy(out=wT[:], in_=w8[:])
                wTs[nt] = wT

            for nt in range(NT_A, min(NT_A + 4, NT)):
                load_w_full(nt)

            # ---- phase A matmuls: k-group-major across NT_A n-tiles
            for g in range(XG):
                for nt in range(NT_A):
                    for kti in range(KGS[g]):
                        kt = KG_START[g] + kti
                        for c in range(MCH):
                            nc.tensor.matmul(
                                psA[nt][c][:],
                                stationary(nt, kt),
                                xTs[g][:, kti, c * MCW:(c + 1) * MCW],
                                start=(kt == 0),
                                stop=(kt == KT - 1),
                            )

            def evict(nt, c, ps):
                o_sb = osb_pool.tile(
                    [P, MCW], mybir.dt.float32, name=f"osb{nt}_{c}", tag="o_sb"
                )
                nc.vector.tensor_scalar(
                    out=o_sb[:],
                    in0=ps[:],
                    scalar1=s_sb[:, nt:nt + 1],
                    scalar2=b_sb[:, nt:nt + 1],
                    op0=mybir.AluOpType.mult,
                    op1=mybir.AluOpType.add,
                )
                nc.sync.dma_start(
                    out=o_d.ap()[nt * P:(nt + 1) * P, c * MCW:(c + 1) * MCW],
                    in_=o_sb[:],
                )

            for nt in range(NT_A):
                for c in range(MCH):
                    evict(nt, c, psA[nt][c])

            # ---- phase B: remaining n-tiles, k-inner / ch-inner
            for nt in range(NT_A, NT):
                ps = [
                    psum_pool.tile(
                        [P, MCW], mybir.dt.float32, name=f"ps{nt}_{c}", tag="ps"
                    )
                    for c in range(MCH)
                ]
                for kt in range(KT):
                    g, kti = _group_of(kt)
                    for c in range(MCH):
                        nc.tensor.matmul(
                            ps[c][:],
                            wTs[nt][:, kt, :],
                            xTs[g][:, kti, c * MCW:(c + 1) * MCW],
                            start=(kt == 0),
                            stop=(kt == KT - 1),
                        )
                if nt + 4 < NT:
                    load_w_full(nt + 4)
                for c in range(MCH):
                    evict(nt, c, ps[c])

    nc.compile()
    return nc


def make_in_maps(x, weight_quant, scale, bias):
    x2T = np.ascontiguousarray(
        np.asarray(x, dtype=np.float32).reshape(M, K).T
    )  # [K, M] k-major replica
    wq = np.asarray(weight_quant, dtype=np.int32).astype(np.int8)  # int8-valued
    scale = np.asarray(scale, dtype=np.float32)
    bias = np.asarray(bias, dtype=np.float32)
    in_maps = []
    for i in range(NCORES):
        sl = slice(i * NSH, (i + 1) * NSH)
        # [nsh, k] -> [nt, n, kt, p] -> [nt, p, kt, n] -> [nt*p, kt, n]
        w_sw = np.ascontiguousarray(
            wq[sl].reshape(NT, P, KT, P).transpose(0, 3, 2, 1)
        ).reshape(NT * P, KT, P)
        in_maps.append({
            "xT": x2T,
            "wq": w_sw,
            "scale": np.ascontiguousarray(scale[sl].reshape(NT, P).T),
            "bias": np.ascontiguousarray(bias[sl].reshape(NT, P).T),
        })
    return in_maps


def gather_output(results):
    outT = np.concatenate([np.asarray(r["outT"]) for r in results], axis=0)  # [N, M]
    return np.ascontiguousarray(outT.T).reshape(B, S, N).astype(np.float32, copy=False)


def kernel(x, weight_quant, scale, bias):
    nc = build()
    in_maps = make_in_maps(x, weight_quant, scale, bias)
    res = run_bass_kernel_spmd(nc, in_maps, core_ids=list(range(NCORES)))
    return gather_output(res.results)


if __name__ == "__main__":
    rng = np.random.default_rng(0)
    x = rng.standard_normal((B, S, K), dtype=np.float32)
    wq = rng.integers(-128, 128, size=(N, K), dtype=np.int64).astype(np.int32)
    scale = rng.uniform(0.001, 0.02, size=(N,)).astype(np.float32)
    bias = rng.standard_normal((N,), dtype=np.float32)
    out = kernel(x=x, weight_quant=wq, scale=scale, bias=bias)
    w = wq.astype(np.float32) * scale[:, None]
    exp = x.reshape(M, K) @ w.T + bias
    err = np.abs(out.reshape(M, N) - exp).max() / np.abs(exp).max()
    print("self-check rel err:", err)
